# revision 30
# baseline (speedup 1.0000x reference)
"""Trainium2 Bass kernel for nn_MemristorCNN (embedding_lookup, 8 cores).

Strategy:
- Host gathers W1 = values[w_idx1], shards it column-wise over in_features
  (4 conv2 output channels per core), permutes columns to the device's
  chunked feature order and pre-tiles to [128, 100, 512] bf16 so the
  weight streams as 4 big contiguous SWDGE DMAs on the (otherwise idle)
  GpSimd queue, overlapping the whole conv stack.
- Conv stack is data-parallel (4 images/core).  conv1 packs
  (half, img, dy, dx) into K=72 with half in a contiguous 64-partition
  block so the pool1->conv2 repack is 8 large block DMAs.  conv2 packs
  (dx-pair, img, ch) into K=128 with 6 tap passes.
- PSUM evacuation does relu (+bias) first (fp32 PSUM -> bf16 SBUF,
  split between Scalar and Vector by a balance ratio), then 2x2 maxpool
  as two DVE tensor-max ops in bf16 2x mode (w pairs deinterleaved by
  the evacuation AP so operands are step-1).
- AllToAll runs in 2 row-chunks (rows 0:32, 32:56 + pad), each received
  buffer is [img, ch, pix] contiguous so one dma_start_transpose yields
  the feature-major fc1 operand; fc1 weight columns are host-permuted to
  the resulting (partition-major) feature order.  fc1 accumulates one
  PSUM group over 100 k-tiles; ReduceScatter + fc2 finish; host concats
  per-core [4, 4] outputs.
"""

import sys

import numpy as np
import ml_dtypes

BF16NP = ml_dtypes.bfloat16

for _p in ("/opt/trn_rl_repo",):
    if _p not in sys.path:
        sys.path.insert(0, _p)

import concourse.bacc as bacc
import concourse.bass as bass  # noqa: F401
import concourse.tile as tile
from concourse import mybir
from concourse.bass_utils import run_bass_kernel_spmd

F32 = mybir.dt.float32
BF16 = mybir.dt.bfloat16
RELU = mybir.ActivationFunctionType.Relu
COPY = mybir.ActivationFunctionType.Copy
ADD = mybir.AluOpType.add
MAX = mybir.AluOpType.max

N_CORES = 8
B = 32
IMG = 224
C1, C2 = 16, 32
PH, PW = 112, 112
HH, HW = 56, 56
FEAT = C2 * HH * HW          # 100352
FSH = FEAT // N_CORES        # 12544 = 4 ch * 3136 px
H1 = 512
NOUT = 4

# a2a row chunks (pooled rows 0:16, 16:36, 36:56) -> pix spans below;
# each chunk's 4ch*pix is a multiple of 128, so k-tiles pack exactly.
PIX = (896, 1120, 1120)          # px per channel per chunk
PIX0 = (0, 896, 2016)            # channel-relative px offset
CHT = (4, 9, 14)                 # conv2 T index after which chunk closes
NKC = tuple(4 * p // 128 for p in PIX)   # (28, 35, 35)
K0 = (0, NKC[0], NKC[0] + NKC[1], 98)
NK = 98
W_CHUNKS = 4

_CACHE = {}


def _build_program(stop_after: str = 'full'):
    nc = bacc.Bacc("TRN2", target_bir_lowering=False, debug=False,
                   num_devices=N_CORES)
    _emit(nc, stop_after)
    nc.compile()
    return nc


def _emit(nc, stop_after: str):
    # ---- kernel I/O ----
    x9_t = nc.dram_tensor("x9", [72, PH, 232], BF16, kind="ExternalInput")
    s1_t = nc.dram_tensor("s1", [72, 128], BF16, kind="ExternalInput")
    s2_t = nc.dram_tensor("s2", [128, 6, 128], BF16, kind="ExternalInput")
    w1t_t = nc.dram_tensor("w1t", [128, NK, H1], BF16, kind="ExternalInput")
    # packed small consts: col 0 cb1, 1 cb2, 2:18 b1t, 18:34 w2t, 34:38 b2t
    cst_t = nc.dram_tensor("cst", [128, 38], F32, kind="ExternalInput")
    out_t = nc.dram_tensor("out", [4, NOUT], F32, kind="ExternalOutput")

    # ---- internal DRAM (collective bounce buffers) ----
    a2a_ins = [nc.dram_tensor(f"a2a_in{i}", [B, 4, PIX[i]], BF16)
               for i in range(3)]
    a2a_outs = [nc.dram_tensor(f"a2a_out{i}", [B, 4, PIX[i]], BF16)
                for i in range(3)]
    rs_in = nc.dram_tensor("rs_in", [B, H1], F32)
    rs_out = nc.dram_tensor("rs_out", [4, H1], F32)
    warm_in = nc.dram_tensor("warm_in", [8, 64], BF16)
    warm_out = nc.dram_tensor("warm_out", [8, 64], BF16)

    groups = [list(range(N_CORES))]

    with tile.TileContext(nc) as tc:
        with (
            tc.tile_pool(name="wgt", bufs=1) as wgt,
            tc.tile_pool(name="const", bufs=1) as cpool,
            tc.tile_pool(name="pers", bufs=1) as pers,
            tc.tile_pool(name="xq", bufs=2) as xq,
            tc.tile_pool(name="ev", bufs=2) as ev,
            tc.tile_pool(name="wk", bufs=2) as wk,
            tc.tile_pool(name="cps", bufs=2, space="PSUM") as cps,
            tc.tile_pool(name="fps", bufs=1, space="PSUM") as fps,
            tc.tile_pool(name="f2ps", bufs=1, space="PSUM") as f2ps,
        ):
            # -------- latency-critical loads first --------
            # small consts first (conv1 stationaries), then the two x9
            # halves on the two separate HWDGE rings in parallel.
            s1_sb = cpool.tile([72, 128], BF16, tag="s1")
            nc.scalar.dma_start(out=s1_sb[:, :], in_=s1_t[:, :])
            s2_sb = cpool.tile([128, 6, 128], BF16, tag="s2")
            nc.scalar.dma_start(out=s2_sb[:, :, :], in_=s2_t[:, :, :])
            cst_sb = cpool.tile([128, 38], F32, tag="cst")
            nc.scalar.dma_start(out=cst_sb[:, :], in_=cst_t[:, :])
            x9_tiles = []
            for q in range(2):
                x9q = xq.tile([72, 56, 232], BF16, tag="x9")
                eng = nc.scalar if q == 0 else nc.sync
                eng.dma_start(out=x9q[:, :, :],
                              in_=x9_t[:, 56 * q:56 * q + 56, :])
                x9_tiles.append(x9q)

            # PE warm-up: garbage matmuls so the HAM clock-gate opens
            # before conv1's first real matmul (and stays open).
            junk = cpool.tile([72, 512], BF16, tag="junk")
            nc.vector.memset(junk[:, :], 0.0)
            warm_psa = fps.tile([B, H1], F32, tag="fc1psa")
            warm_psb = fps.tile([B, H1], F32, tag="fc1psb")
            for i in range(32):
                wp = warm_psa if i % 2 == 0 else warm_psb
                nc.tensor.matmul(wp[:, :], lhsT=s1_sb[:, 0:32],
                                 rhs=junk[:, :], start=True, stop=True)
            cb1_sb = cst_sb[:, 0:1]
            cb2_sb = cst_sb[:, 1:2]
            b1t_sb = cst_sb[:, 2:18].rearrange("p (k i) -> p k i", k=4)
            w2t_sb = cst_sb[:, 18:34].rearrange("p (k o) -> p k o", k=4)
            b2t_sb = cst_sb[0:4, 34:38]

            # warm up the collective path so the first real a2a runs at
            # full rate (the first collective pays ~20us of setup)
            warm_sb = wk.tile([8, 64], BF16, tag="warm")
            nc.gpsimd.memset(warm_sb[:, :], 0.0)
            nc.gpsimd.dma_start(out=warm_in[:, :], in_=warm_sb[:, :])
            nc.gpsimd.collective_compute(
                "AllToAll", mybir.AluOpType.bypass, replica_groups=groups,
                ins=[warm_in[:, :]], outs=[warm_out[:, :]])

            # conv2 input: partition e*64 + img*16 + ch; rows 1+g (g = global
            # pooled conv1 row), dx-shifted by e.  Zero only the halo border.
            c2in = pers.tile([128, 114, 116], BF16, tag="c2in")
            nc.gpsimd.memset(c2in[:, 0, :], 0.0)
            nc.gpsimd.memset(c2in[:, 113, :], 0.0)
            nc.gpsimd.memset(c2in[0:64, :, 0:1], 0.0)
            nc.gpsimd.memset(c2in[0:64, :, 113:116], 0.0)
            nc.gpsimd.memset(c2in[64:128, :, 112:116], 0.0)

            # -------- fc1 weight stream on GpSimd (SWDGE), 4 big chunks ----
            # (held behind x9 q0/q1 arrival so the input loads win the HBM
            # bandwidth race at startup)
            w_sb = wgt.tile([128, NK, H1], BF16, tag="w1")
            wgate = wk.tile([1, 8], F32, tag="wgate")
            nc.gpsimd.tensor_copy(wgate[:, 0:1], x9_tiles[0][0:1, 0, 0:1])
            nc.gpsimd.tensor_copy(wgate[:, 1:2], x9_tiles[1][0:1, 0, 0:1])
            nc.gpsimd.tensor_copy(wgate[:, 2:3], s1_sb[0:1, 0:1])
            nc.gpsimd.tensor_copy(wgate[:, 3:4], s2_sb[0:1, 0, 0:1])
            nc.gpsimd.tensor_copy(wgate[:, 4:5], cst_sb[0:1, 0:1])
            kb = [0, 25, 50, 75, NK]
            for ci in range(W_CHUNKS):
                nc.gpsimd.dma_start(out=w_sb[:, kb[ci]:kb[ci + 1], :],
                                    in_=w1t_t[:, kb[ci]:kb[ci + 1], :])

            # ---------------- conv1 + relu + pool1 ----------------
            # out partition m = half*64 + img*16 + oc.  The matmul writes
            # PSUM through a strided AP so the psum layout is [g, r, w, x]
            # (pool pairs deinterleaved); evacuation is then a contiguous
            # copy and the pool maxes run in DVE 2x bf16 mode.  The pooled
            # output goes straight into c2in: half0 -> e0 block, half1 ->
            # e1 block (same partitions); the two cross copies are DMAs.

            def evac(out_c, ps_v, bias, use_vector):
                # relu(+bias): fp32 PSUM -> bf16 SBUF, contiguous
                if use_vector:
                    nc.vector.tensor_scalar(out_c, ps_v, bias, 0.0,
                                            op0=ADD, op1=MAX)
                else:
                    nc.scalar.activation(out_c, ps_v, RELU, bias=bias)

            for T in range(28):            # 2 pooled rows per psum tile
                ps = cps.tile([128, 2, 512], F32, tag="cps")
                for g in range(2):
                    yp = T * 2 + g         # pooled row within half
                    q, ypl = yp // 28, yp % 28
                    nc.tensor.matmul(
                        ps[:, g, 0:448],
                        lhsT=s1_sb[:, :],
                        rhs=x9_tiles[q][:, 2 * ypl:2 * ypl + 2, :224],
                        start=True, stop=True)
                # evac reads psum via a w-deinterleaving view; c1 is written
                # contiguous as [w, g, r, x]
                c1 = ev.tile([128, 2, 2, 2, 112], BF16, tag="c1")
                evac(c1[:, :, :, :, :],
                     ps[:, :, 0:448].rearrange("p g (r x w) -> p w g r x",
                                               r=2, w=2),
                     cb1_sb, use_vector=(T % 14 >= 11))
                m1 = ev.tile([128, 2, 2, 112], BF16, tag="m1")     # [g,r,x]
                nc.vector.tensor_max(m1[:, :, :, :], c1[:, 0, :, :, :],
                                     c1[:, 1, :, :, :])
                # pooled rows 2T, 2T+1: half0 -> c2in e0 rows 1+2T,
                # half1 -> c2in e1 rows 57+2T
                nc.vector.tensor_max(
                    c2in[0:64, 1 + 2 * T:3 + 2 * T, 1:113],
                    m1[0:64, :, 0, :], m1[0:64, :, 1, :])
                nc.vector.tensor_max(
                    c2in[64:128, 57 + 2 * T:59 + 2 * T, 0:112],
                    m1[64:128, :, 0, :], m1[64:128, :, 1, :])

                if T == 13 or T == 27:
                    # cross copies for rows chunk (28 rows each half):
                    # half0 block (e0, rows 1..57) -> e1 partitions;
                    # half1 block (e1, rows 57..113) -> e0 partitions.
                    r0 = 1 + 28 * (T // 14)
                    r1 = 57 + 28 * (T // 14)
                    nc.sync.dma_start(
                        out=c2in[64:128, r0:r0 + 28, 0:112],
                        in_=c2in[0:64, r0:r0 + 28, 1:113])
                    nc.sync.dma_start(
                        out=c2in[0:64, r1:r1 + 28, 1:113],
                        in_=c2in[64:128, r1:r1 + 28, 0:112])

            if stop_after == "conv1":
                dbg = wk.tile([4, NOUT], F32, tag="outsb")
                nc.vector.tensor_copy(dbg[:, :], c2in[0:4, 1, 1:5])
                nc.sync.dma_start(out=out_t[:, :], in_=dbg[:, :])
                return

            # ---------------- conv2 + relu + pool2 ----------------
            # out partition m = img*32 + oc; 6 passes t=(dy, grp):
            # partition block e supplies tap dx = 2*grp + e.
            h_sb = pers.tile([128, 14, 4, 56], BF16, tag="hsb")  # [T, row, x]
            for T in range(14):            # 8 conv rows / 4 pooled rows
                ps = cps.tile([128, 2, 512], F32, tag="cps")
                for t in range(6):
                    dy, grp = t // 2, t % 2
                    for sub in range(2):   # alternate psum banks
                        y0 = 8 * T + 4 * sub
                        nc.tensor.matmul(
                            ps[:, sub, 0:448],
                            lhsT=s2_sb[:, t, :],
                            rhs=c2in[:, y0 + dy:y0 + dy + 4,
                                     2 * grp:2 * grp + 112],
                            start=(t == 0), stop=(t == 5))
                c1 = ev.tile([128, 2, 2, 4, 56], BF16, tag="c2c1")  # [w,s,r,x]
                evac(c1[:, :, :, :, :],
                     ps[:, :, 0:448].rearrange("p s (r x w) -> p w s r x",
                                               r=4, w=2),
                     cb2_sb, use_vector=(T % 14 >= 11))
                m1 = ev.tile([128, 2, 4, 56], BF16, tag="c2m1")     # [s,r,x]
                nc.vector.tensor_max(m1[:, :, :, :], c1[:, 0, :, :, :],
                                     c1[:, 1, :, :, :])
                v2 = m1[:, :, :, :].rearrange("p s (rp rw) x -> p s rp rw x",
                                              rw=2)
                nc.vector.tensor_max(h_sb[:, T, :, :].rearrange(
                    "p (s rp) x -> p s rp x", s=2),
                    v2[:, :, :, 0, :], v2[:, :, :, 1, :])

                # -------- a2a chunks fire as their rows complete --------
                # h partition m = 16d + 4j + c  <->  dst row (4d+j) ch c,
                # so each write is a plain contiguous 2D copy.
                if T + 1 in CHT:
                    i = CHT.index(T + 1)
                    Ta, Tb = ((0,) + CHT)[i], CHT[i]
                    nc.sync.dma_start(
                        out=a2a_ins[i][:, :, :].rearrange(
                            "r c x -> (r c) x"),
                        in_=h_sb[:, Ta:Tb, :, :].rearrange(
                            "p a b x -> p (a b x)"))
                    nc.gpsimd.collective_compute(
                        "AllToAll", mybir.AluOpType.bypass,
                        replica_groups=groups,
                        ins=[a2a_ins[i][:, :, :]],
                        outs=[a2a_outs[i][:, :, :]])

            if stop_after == "conv2":
                dbg = wk.tile([4, NOUT], F32, tag="outsb")
                nc.vector.tensor_copy(dbg[:, :], h_sb[0:4, 0, 0, 0:4])
                nc.sync.dma_start(out=out_t[:, :], in_=dbg[:, :])
                return

            # -------- transpose received chunks to feature-major --------
            # hT[p, k, i] = chunk feature (k*128 + p), image i
            hT = pers.tile([128, NK, B], BF16, tag="hT")
            for i in range(3):
                nc.scalar.dma_start_transpose(
                    out=hT[:, K0[i]:K0[i + 1], :],
                    in_=a2a_outs[i][:, :, :].rearrange("i c x -> i (c x)"))

            if stop_after == "a2a":
                dbg = wk.tile([4, NOUT], F32, tag="outsb")
                nc.vector.tensor_copy(dbg[:, :], hT[0:4, 0, 0:4])
                nc.sync.dma_start(out=out_t[:, :], in_=dbg[:, :])
                return

            # ---------------- fc1 partial ----------------
            # two psum banks, alternated per k-tile so consecutive matmuls
            # never chain into the same bank (fill/drain overlap)
            fc1_psa = fps.tile([B, H1], F32, tag="fc1psa")
            fc1_psb = fps.tile([B, H1], F32, tag="fc1psb")
            for k in range(NK):
                pp = fc1_psa if k % 2 == 0 else fc1_psb
                nc.tensor.matmul(pp[:, :], lhsT=hT[:, k, :],
                                 rhs=w_sb[:, k, :],
                                 start=(k < 2), stop=(k >= NK - 2))
            fc1_sb = wk.tile([B, H1], F32, tag="fc1")
            nc.scalar.activation(fc1_sb[:, :], fc1_psa[:, :], COPY)
            nc.vector.tensor_add(fc1_sb[:, :], fc1_sb[:, :], fc1_psb[:, :])
            nc.sync.dma_start(out=rs_in[:, :], in_=fc1_sb[:, :])

            if stop_after == "fc1":
                nc.sync.dma_start(out=out_t[:, :], in_=fc1_sb[0:4, 0:4])
                return

            # -------- ReduceScatter + bias + relu + fc2 --------
            nc.gpsimd.collective_compute(
                "ReduceScatter", mybir.AluOpType.add, replica_groups=groups,
                ins=[rs_in[:, :]], outs=[rs_out[:, :]])

            h2t = wk.tile([128, 4, 4], F32, tag="h2t")   # [c, k, img]
            for k in range(4):
                nc.sync.dma_start(
                    out=h2t[:, k, :],
                    in_=rs_out[:, 128 * k:128 * k + 128].rearrange(
                        "i p -> p i"))
            nc.vector.tensor_add(h2t[:, :, :], h2t[:, :, :], b1t_sb)
            nc.scalar.activation(h2t[:, :, :], h2t[:, :, :], RELU)

            fc2_ps = f2ps.tile([4, 4], F32, tag="fc2ps")
            for k in range(4):
                nc.tensor.matmul(fc2_ps[:, :], lhsT=h2t[:, k, :],
                                 rhs=w2t_sb[:, k, :],
                                 start=(k == 0), stop=(k == 3))
            out_sb = wk.tile([4, NOUT], F32, tag="outsb")
            nc.vector.tensor_add(out_sb[:, :], fc2_ps[:, :], b2t_sb)
            nc.sync.dma_start(out=out_t[:, :], in_=out_sb[:, :])


def _get_program(stop_after: str = 'full'):
    key = ("prog_v2", stop_after)
    if key not in _CACHE:
        _CACHE[key] = _build_program(stop_after)
    return _CACHE[key]


def _feature_map():
    """Local (core-relative) original feature index for tiled weight slot
    (p, k).  Original local feature = cc*3136 + pix, pix = y*56 + x.
    Chunk i (K0[i] <= k < K0[i+1]): l = (k - K0[i])*128 + p =
    cc*PIX[i] + pos, pix = PIX0[i] + pos."""
    p = np.arange(128)[:, None]
    k = np.arange(NK)[None, :]
    m = np.empty((128, NK), np.int64)
    for i in range(3):
        kk = k[:, K0[i]:K0[i + 1]]
        l = (kk - K0[i]) * 128 + p
        cc, pos = l // PIX[i], l % PIX[i]
        m[:, K0[i]:K0[i + 1]] = cc * 3136 + PIX0[i] + pos
    return m


_FMAP = _feature_map()


def _host_prep(x, conv1_w, conv1_b, conv2_w, conv2_b, values, w_idx1,
               fc1_b, w_idx2, fc2_b):
    """Build per-core input maps (numpy, bf16 for PE-facing tensors)."""
    f32 = np.float32
    x = np.asarray(x, f32)
    conv1_w = np.asarray(conv1_w, f32)
    conv2_w = np.asarray(conv2_w, f32)
    values = np.asarray(values, f32)
    w_idx1 = np.asarray(w_idx1)
    w_idx2 = np.asarray(w_idx2)

    x_pad = np.zeros((B, 226, 232), f32)
    x_pad[:, 1:225, 1:225] = x[:, 0]

    # x9[c]: [72, 112, 232]; partition (dy*3+dx)*8 + h, h = half*4 + img_loc
    x9 = np.zeros((N_CORES, 72, PH, 232), f32)
    for dy in range(3):
        for dx in range(3):
            for h in range(8):
                half, il = h // 4, h % 4
                y0 = PH * half
                for c in range(N_CORES):
                    x9[c, (dy * 3 + dx) * 8 + h, :, :232 - dx] = \
                        x_pad[4 * c + il, y0 + dy:y0 + dy + PH, dx:]

    s1 = np.zeros((72, 128), f32)
    for dy in range(3):
        for dx in range(3):
            for h in range(8):
                s1[(dy * 3 + dx) * 8 + h, 16 * h:16 * h + C1] = \
                    conv1_w[:, 0, dy, dx]

    # conv2 stationaries [128, 6, 128]: pass t = dy*2 + grp;
    # partition p = e*64 + img*16 + ch supplies tap dx = 2*grp + e.
    # out column m = (oc//4)*16 + img*4 + oc%4 (a2a-friendly order).
    s2 = np.zeros((128, 6, 128), f32)
    for t in range(6):
        dy, grp = t // 2, t % 2
        for e in range(2):
            dx = 2 * grp + e
            if dx > 2:
                continue
            for img in range(4):
                for ch in range(C1):
                    for oc in range(C2):
                        s2[64 * e + 16 * img + ch, t,
                           (oc // 4) * 16 + img * 4 + oc % 4] = \
                            conv2_w[oc, ch, dy, dx]

    # fc1 weight: per core gather + permute to tiled feature order
    w1ts = []
    for c in range(N_CORES):
        idx = w_idx1[:, FSH * c:FSH * (c + 1)]          # [512, 12544]
        wcols = values[idx].T                            # [12544, 512]
        w1ts.append(np.ascontiguousarray(
            wcols[_FMAP]).astype(BF16NP))                # [128, NK, 512]

    # packed consts [128, 38]: 0 cb1, 1 cb2, 2:18 b1t (k-major), 18:34 w2t
    # (w2t[p, k, o] = W2.T[k*128+p, o]), 34:38 b2t (partitions 0..3)
    cst = np.zeros((128, 38), f32)
    for h in range(8):
        cst[16 * h:16 * h + C1, 0] = np.asarray(conv1_b, f32)
    c2b = np.asarray(conv2_b, f32)
    for img in range(4):
        for oc in range(C2):
            cst[(oc // 4) * 16 + img * 4 + oc % 4, 1] = c2b[oc]
    cst[:, 2:18] = np.repeat(np.asarray(fc1_b, f32).reshape(4, 128).T,
                             4, axis=1)
    w2tT = values[w_idx2].T.astype(f32)                  # [512, 4]
    cst[:, 18:34] = w2tT.reshape(4, 128, 4).transpose(1, 0, 2).reshape(128, 16)
    cst[:, 34:38] = np.asarray(fc2_b, f32)[None, :]

    s1 = s1.astype(BF16NP)
    s2 = s2.astype(BF16NP)
    in_maps = []
    for c in range(N_CORES):
        in_maps.append({
            "x9": np.ascontiguousarray(x9[c]).astype(BF16NP),
            "s1": s1, "s2": s2,
            "w1t": w1ts[c],
            "cst": cst,
        })
    return in_maps


def kernel(x, conv1_w, conv1_b, conv2_w, conv2_b, values, w_idx1, fc1_b,
           w_idx2, fc2_b, _trace=False, _trace_kwargs=None,
           _stop_after='full'):
    nc = _get_program(_stop_after)
    in_maps = _host_prep(x, conv1_w, conv1_b, conv2_w, conv2_b, values,
                         w_idx1, fc1_b, w_idx2, fc2_b)
    res = run_bass_kernel_spmd(nc, in_maps, core_ids=list(range(N_CORES)),
                               trace=_trace, **(_trace_kwargs or {}))
    out = np.zeros((B, NOUT), np.float32)
    for c in range(N_CORES):
        out[4 * c:4 * c + 4] = res.results[c]["out"]
    if _trace:
        kernel.last_result = res
    return out


if __name__ == "__main__":
    rng = np.random.default_rng(0)
    ins = {
        "x": rng.standard_normal((B, 1, IMG, IMG), dtype=np.float32),
        "conv1_w": rng.standard_normal((16, 1, 3, 3), dtype=np.float32) * 0.1,
        "conv1_b": np.zeros(16, np.float32),
        "conv2_w": rng.standard_normal((32, 16, 3, 3), dtype=np.float32) * 0.05,
        "conv2_b": np.zeros(32, np.float32),
        "values": np.sort(rng.standard_normal(4096).astype(np.float32) * 0.01),
        "w_idx1": rng.integers(0, 4096, (512, FEAT), dtype=np.int32),
        "fc1_b": np.zeros(512, np.float32),
        "w_idx2": rng.integers(0, 4096, (4, 512), dtype=np.int32),
        "fc2_b": np.zeros(4, np.float32),
    }
    out = kernel(**ins)
    print("out shape", out.shape, "sample row", out[0])


# revision 33
# speedup vs baseline: 1.4999x; 1.4999x over previous
"""Trainium2 Bass kernel for nn_MemristorCNN (embedding_lookup, 8 cores).

Strategy:
- Host gathers W1 = values[w_idx1], shards it column-wise over in_features
  (4 conv2 output channels per core), permutes columns to the device's
  chunked feature order and pre-tiles to [128, 100, 512] bf16 so the
  weight streams as 4 big contiguous SWDGE DMAs on the (otherwise idle)
  GpSimd queue, overlapping the whole conv stack.
- Conv stack is data-parallel (4 images/core).  conv1 packs
  (half, img, dy, dx) into K=72 with half in a contiguous 64-partition
  block so the pool1->conv2 repack is 8 large block DMAs.  conv2 packs
  (dx-pair, img, ch) into K=128 with 6 tap passes.
- PSUM evacuation does relu (+bias) first (fp32 PSUM -> bf16 SBUF,
  split between Scalar and Vector by a balance ratio), then 2x2 maxpool
  as two DVE tensor-max ops in bf16 2x mode (w pairs deinterleaved by
  the evacuation AP so operands are step-1).
- AllToAll runs in 2 row-chunks (rows 0:32, 32:56 + pad), each received
  buffer is [img, ch, pix] contiguous so one dma_start_transpose yields
  the feature-major fc1 operand; fc1 weight columns are host-permuted to
  the resulting (partition-major) feature order.  fc1 accumulates one
  PSUM group over 100 k-tiles; ReduceScatter + fc2 finish; host concats
  per-core [4, 4] outputs.
"""

import sys

import numpy as np
import ml_dtypes

BF16NP = ml_dtypes.bfloat16

for _p in ("/opt/trn_rl_repo",):
    if _p not in sys.path:
        sys.path.insert(0, _p)

import concourse.bacc as bacc
import concourse.bass as bass  # noqa: F401
import concourse.tile as tile
from concourse import mybir
from concourse.bass_utils import run_bass_kernel_spmd

F32 = mybir.dt.float32
FP8 = mybir.dt.float8e4
FP8NP = ml_dtypes.float8_e4m3
WSCALE = 256.0
BF16 = mybir.dt.bfloat16
RELU = mybir.ActivationFunctionType.Relu
COPY = mybir.ActivationFunctionType.Copy
ADD = mybir.AluOpType.add
MAX = mybir.AluOpType.max

N_CORES = 8
B = 32
IMG = 224
C1, C2 = 16, 32
PH, PW = 112, 112
HH, HW = 56, 56
FEAT = C2 * HH * HW          # 100352
FSH = FEAT // N_CORES        # 12544 = 4 ch * 3136 px
H1 = 512
NOUT = 4

# a2a row chunks (pooled rows 0:16, 16:36, 36:56) -> pix spans below;
# each chunk's 4ch*pix is a multiple of 128, so k-tiles pack exactly.
PIX = (896, 1120, 1120)          # px per channel per chunk
PIX0 = (0, 896, 2016)            # channel-relative px offset
CHT = (4, 9, 14)                 # conv2 T index after which chunk closes
NKC = tuple(4 * p // 128 for p in PIX)   # (28, 35, 35)
K0 = (0, NKC[0], NKC[0] + NKC[1], 98)
NK = 98
W_CHUNKS = 4

_CACHE = {}


def _build_program(stop_after: str = 'full'):
    nc = bacc.Bacc("TRN2", target_bir_lowering=False, debug=False,
                   num_devices=N_CORES)
    _emit(nc, stop_after)
    nc.compile()
    return nc


def _emit(nc, stop_after: str):
    # ---- kernel I/O ----
    x9_t = nc.dram_tensor("x9", [72, PH, 224], BF16, kind="ExternalInput")
    s1_t = nc.dram_tensor("s1", [72, 128], BF16, kind="ExternalInput")
    s2_t = nc.dram_tensor("s2", [128, 6, 128], BF16, kind="ExternalInput")
    w1t_t = nc.dram_tensor("w1t", [128, NK, H1], FP8, kind="ExternalInput")
    # packed small consts: col 0 cb1, 1 cb2, 2:18 b1t, 18:34 w2t, 34:38 b2t
    cst_t = nc.dram_tensor("cst", [128, 38], F32, kind="ExternalInput")
    out_t = nc.dram_tensor("out", [4, NOUT], F32, kind="ExternalOutput")

    # ---- internal DRAM (collective bounce buffers) ----
    a2a_ins = [nc.dram_tensor(f"a2a_in{i}", [B, 4, PIX[i]], BF16)
               for i in range(3)]
    a2a_outs = [nc.dram_tensor(f"a2a_out{i}", [B, 4, PIX[i]], BF16)
                for i in range(3)]
    rs_in = nc.dram_tensor("rs_in", [B, H1], F32)
    rs_out = nc.dram_tensor("rs_out", [4, H1], F32)
    warm_in = nc.dram_tensor("warm_in", [8, 64], BF16)
    warm_out = nc.dram_tensor("warm_out", [8, 64], BF16)

    groups = [list(range(N_CORES))]

    with tile.TileContext(nc) as tc:
        with (
            tc.tile_pool(name="wgt", bufs=1) as wgt,
            tc.tile_pool(name="const", bufs=1) as cpool,
            tc.tile_pool(name="pers", bufs=1) as pers,
            tc.tile_pool(name="xq", bufs=2) as xq,
            tc.tile_pool(name="ev", bufs=2) as ev,
            tc.tile_pool(name="wk", bufs=2) as wk,
            tc.tile_pool(name="cps", bufs=2, space="PSUM") as cps,
            tc.tile_pool(name="fps", bufs=1, space="PSUM") as fps,
            tc.tile_pool(name="f2ps", bufs=1, space="PSUM") as f2ps,
        ):
            # -------- latency-critical loads first --------
            # small consts first (conv1 stationaries), then the two x9
            # halves on the two separate HWDGE rings in parallel.
            s1_sb = cpool.tile([72, 128], BF16, tag="s1")
            nc.scalar.dma_start(out=s1_sb[:, :], in_=s1_t[:, :])
            s2_sb = cpool.tile([128, 6, 128], BF16, tag="s2")
            nc.scalar.dma_start(out=s2_sb[:, :, :], in_=s2_t[:, :, :])
            cst_sb = cpool.tile([128, 38], F32, tag="cst")
            nc.scalar.dma_start(out=cst_sb[:, :], in_=cst_t[:, :])
            x9_tiles = []
            for q in range(2):
                x9q = xq.tile([72, 56, 224], BF16, tag="x9")
                eng = nc.scalar if q == 0 else nc.sync
                eng.dma_start(out=x9q[:, :, :],
                              in_=x9_t[:, 56 * q:56 * q + 56, :])
                x9_tiles.append(x9q)

            # PE warm-up: garbage matmuls so the HAM clock-gate opens
            # before conv1's first real matmul (and stays open).
            junk = cpool.tile([72, 512], BF16, tag="junk")
            nc.vector.memset(junk[:, :], 0.0)
            warm_psa = fps.tile([B, H1], F32, tag="fc1psa")
            warm_psb = fps.tile([B, H1], F32, tag="fc1psb")
            for i in range(32):
                wp = warm_psa if i % 2 == 0 else warm_psb
                nc.tensor.matmul(wp[:, :], lhsT=s1_sb[:, 0:32],
                                 rhs=junk[:, :], start=True, stop=True)
            cb1_sb = cst_sb[:, 0:1]
            cb2_sb = cst_sb[:, 1:2]
            b1t_sb = cst_sb[:, 2:18].rearrange("p (k i) -> p k i", k=4)
            w2t_sb = cst_sb[:, 18:34].rearrange("p (k o) -> p k o", k=4)
            b2t_sb = cst_sb[0:4, 34:38]

            # warm up the collective path so the first real a2a runs at
            # full rate (the first collective pays ~20us of setup)
            warm_sb = wk.tile([8, 64], BF16, tag="warm")
            nc.gpsimd.memset(warm_sb[:, :], 0.0)
            nc.gpsimd.dma_start(out=warm_in[:, :], in_=warm_sb[:, :])
            nc.gpsimd.collective_compute(
                "AllToAll", mybir.AluOpType.bypass, replica_groups=groups,
                ins=[warm_in[:, :]], outs=[warm_out[:, :]])

            # conv2 input: partition e*64 + img*16 + ch; rows 1+g (g = global
            # pooled conv1 row), dx-shifted by e.  Zero only the halo border.
            c2in = pers.tile([128, 114, 116], BF16, tag="c2in")
            nc.gpsimd.memset(c2in[:, 0, :], 0.0)
            nc.gpsimd.memset(c2in[:, 113, :], 0.0)
            nc.gpsimd.memset(c2in[0:64, :, 0:1], 0.0)
            nc.gpsimd.memset(c2in[0:64, :, 113:116], 0.0)
            nc.gpsimd.memset(c2in[64:128, :, 112:116], 0.0)

            # -------- fc1 weight stream on GpSimd (SWDGE), 4 big chunks ----
            # (held behind x9 q0/q1 arrival so the input loads win the HBM
            # bandwidth race at startup)
            w_sb = wgt.tile([128, NK, H1], FP8, tag="w1")
            wgate = wk.tile([1, 8], F32, tag="wgate")
            nc.gpsimd.tensor_copy(wgate[:, 0:1], x9_tiles[0][0:1, 0, 0:1])
            nc.gpsimd.tensor_copy(wgate[:, 1:2], x9_tiles[1][0:1, 0, 0:1])
            nc.gpsimd.tensor_copy(wgate[:, 2:3], s1_sb[0:1, 0:1])
            nc.gpsimd.tensor_copy(wgate[:, 3:4], s2_sb[0:1, 0, 0:1])
            nc.gpsimd.tensor_copy(wgate[:, 4:5], cst_sb[0:1, 0:1])
            kb = [0, 25, 50, 75, NK]
            for ci in range(W_CHUNKS):
                # write one element of the chunk's region first (reading the
                # gate) so the big DMA has a true data dependency on the
                # startup loads and cannot be scheduled before them.
                nc.gpsimd.tensor_copy(w_sb[0:1, kb[ci]:kb[ci] + 1, 0:1],
                                      wgate[0:1, ci:ci + 1])
                nc.gpsimd.dma_start(out=w_sb[:, kb[ci]:kb[ci + 1], :],
                                    in_=w1t_t[:, kb[ci]:kb[ci + 1], :])

            # ---------------- conv1 + relu + pool1 ----------------
            # out partition m = half*64 + img*16 + oc.  The matmul writes
            # PSUM through a strided AP so the psum layout is [g, r, w, x]
            # (pool pairs deinterleaved); evacuation is then a contiguous
            # copy and the pool maxes run in DVE 2x bf16 mode.  The pooled
            # output goes straight into c2in: half0 -> e0 block, half1 ->
            # e1 block (same partitions); the two cross copies are DMAs.

            def evac(out_c, ps_v, bias, use_vector):
                # relu(+bias): fp32 PSUM -> bf16 SBUF, contiguous
                if use_vector:
                    nc.vector.tensor_scalar(out_c, ps_v, bias, 0.0,
                                            op0=ADD, op1=MAX)
                else:
                    nc.scalar.activation(out_c, ps_v, RELU, bias=bias)

            for T in range(28):            # 2 pooled rows per psum tile
                ps = cps.tile([128, 2, 512], F32, tag="cps")
                for g in range(2):
                    yp = T * 2 + g         # pooled row within half
                    q, ypl = yp // 28, yp % 28
                    nc.tensor.matmul(
                        ps[:, g, 0:448],
                        lhsT=s1_sb[:, :],
                        rhs=x9_tiles[q][:, 2 * ypl:2 * ypl + 2, :],
                        start=True, stop=True)
                # evac reads psum via a w-deinterleaving view; c1 is written
                # contiguous as [w, g, r, x]
                c1 = ev.tile([128, 2, 2, 2, 112], BF16, tag="c1")
                evac(c1[:, :, :, :, :],
                     ps[:, :, 0:448].rearrange("p g (r x w) -> p w g r x",
                                               r=2, w=2),
                     cb1_sb, use_vector=(T % 14 >= 11))
                m1 = ev.tile([128, 2, 2, 112], BF16, tag="m1")     # [g,r,x]
                nc.vector.tensor_max(m1[:, :, :, :], c1[:, 0, :, :, :],
                                     c1[:, 1, :, :, :])
                # pooled rows 2T, 2T+1: half0 -> c2in e0 rows 1+2T,
                # half1 -> c2in e1 rows 57+2T
                nc.vector.tensor_max(
                    c2in[0:64, 1 + 2 * T:3 + 2 * T, 1:113],
                    m1[0:64, :, 0, :], m1[0:64, :, 1, :])
                nc.vector.tensor_max(
                    c2in[64:128, 57 + 2 * T:59 + 2 * T, 0:112],
                    m1[64:128, :, 0, :], m1[64:128, :, 1, :])

                if T == 13 or T == 27:
                    # cross copies for rows chunk (28 rows each half):
                    # half0 block (e0, rows 1..57) -> e1 partitions;
                    # half1 block (e1, rows 57..113) -> e0 partitions.
                    r0 = 1 + 28 * (T // 14)
                    r1 = 57 + 28 * (T // 14)
                    nc.sync.dma_start(
                        out=c2in[64:128, r0:r0 + 28, 0:112],
                        in_=c2in[0:64, r0:r0 + 28, 1:113])
                    nc.sync.dma_start(
                        out=c2in[0:64, r1:r1 + 28, 1:113],
                        in_=c2in[64:128, r1:r1 + 28, 0:112])

            if stop_after == "conv1":
                dbg = wk.tile([4, NOUT], F32, tag="outsb")
                nc.vector.tensor_copy(dbg[:, :], c2in[0:4, 1, 1:5])
                nc.sync.dma_start(out=out_t[:, :], in_=dbg[:, :])
                return

            # ---------------- conv2 + relu + pool2 ----------------
            # out partition m = img*32 + oc; 6 passes t=(dy, grp):
            # partition block e supplies tap dx = 2*grp + e.
            h_sb = pers.tile([128, 14, 4, 56], BF16, tag="hsb")  # [T, row, x]
            hT = pers.tile([128, NK, B], BF16, tag="hT")
            for T in range(14):            # 8 conv rows / 4 pooled rows
                ps = cps.tile([128, 2, 512], F32, tag="cps")
                for t in range(6):
                    dy, grp = t // 2, t % 2
                    for sub in range(2):   # alternate psum banks
                        y0 = 8 * T + 4 * sub
                        nc.tensor.matmul(
                            ps[:, sub, 0:448],
                            lhsT=s2_sb[:, t, :],
                            rhs=c2in[:, y0 + dy:y0 + dy + 4,
                                     2 * grp:2 * grp + 112],
                            start=(t == 0), stop=(t == 5))
                c1 = ev.tile([128, 2, 2, 4, 56], BF16, tag="c2c1")  # [w,s,r,x]
                evac(c1[:, :, :, :, :],
                     ps[:, :, 0:448].rearrange("p s (r x w) -> p w s r x",
                                               r=4, w=2),
                     cb2_sb, use_vector=(T % 14 >= 11))
                m1 = ev.tile([128, 2, 4, 56], BF16, tag="c2m1")     # [s,r,x]
                nc.vector.tensor_max(m1[:, :, :, :], c1[:, 0, :, :, :],
                                     c1[:, 1, :, :, :])
                v2 = m1[:, :, :, :].rearrange("p s (rp rw) x -> p s rp rw x",
                                              rw=2)
                nc.vector.tensor_max(h_sb[:, T, :, :].rearrange(
                    "p (s rp) x -> p s rp x", s=2),
                    v2[:, :, :, 0, :], v2[:, :, :, 1, :])

                # -------- a2a chunks fire as their rows complete --------
                # h partition m = 16d + 4j + c  <->  dst row (4d+j) ch c,
                # so each write is a plain contiguous 2D copy.  The
                # transpose to feature-major (hT[p, k, i] = chunk feature
                # k*128+p) follows each collective immediately.
                if T + 1 in CHT:
                    i = CHT.index(T + 1)
                    Ta, Tb = ((0,) + CHT)[i], CHT[i]
                    nc.sync.dma_start(
                        out=a2a_ins[i][:, :, :].rearrange(
                            "r c x -> (r c) x"),
                        in_=h_sb[:, Ta:Tb, :, :].rearrange(
                            "p a b x -> p (a b x)"))
                    nc.gpsimd.collective_compute(
                        "AllToAll", mybir.AluOpType.bypass,
                        replica_groups=groups,
                        ins=[a2a_ins[i][:, :, :]],
                        outs=[a2a_outs[i][:, :, :]])
                    nc.sync.dma_start_transpose(
                        out=hT[:, K0[i]:K0[i + 1], :],
                        in_=a2a_outs[i][:, :, :].rearrange(
                            "i c x -> i (c x)"))

            if stop_after == "conv2":
                dbg = wk.tile([4, NOUT], F32, tag="outsb")
                nc.vector.tensor_copy(dbg[:, :], h_sb[0:4, 0, 0, 0:4])
                nc.sync.dma_start(out=out_t[:, :], in_=dbg[:, :])
                return

            if stop_after == "a2a":
                dbg = wk.tile([4, NOUT], F32, tag="outsb")
                nc.vector.tensor_copy(dbg[:, :], hT[0:4, 0, 0:4])
                nc.sync.dma_start(out=out_t[:, :], in_=dbg[:, :])
                return

            # ---------------- fc1 partial ----------------
            # two psum banks, alternated per k-tile so consecutive matmuls
            # never chain into the same bank (fill/drain overlap)
            fc1_psa = fps.tile([B, H1], F32, tag="fc1psa")
            fc1_psb = fps.tile([B, H1], F32, tag="fc1psb")
            for k in range(NK):
                pp = fc1_psa if k % 2 == 0 else fc1_psb
                nc.tensor.matmul(pp[:, :], lhsT=hT[:, k, :],
                                 rhs=w_sb[:, k, :],
                                 start=(k < 2), stop=(k >= NK - 2))
            fc1_sb = wk.tile([B, H1], F32, tag="fc1")
            nc.scalar.activation(fc1_sb[:, :], fc1_psa[:, :], COPY)
            nc.vector.tensor_add(fc1_sb[:, :], fc1_sb[:, :], fc1_psb[:, :])
            nc.sync.dma_start(out=rs_in[:, :], in_=fc1_sb[:, :])

            if stop_after == "fc1":
                nc.sync.dma_start(out=out_t[:, :], in_=fc1_sb[0:4, 0:4])
                return

            # -------- ReduceScatter + bias + relu + fc2 --------
            nc.gpsimd.collective_compute(
                "ReduceScatter", mybir.AluOpType.add, replica_groups=groups,
                ins=[rs_in[:, :]], outs=[rs_out[:, :]])

            h2t = wk.tile([128, 4, 4], F32, tag="h2t")   # [c, k, img]
            for k in range(4):
                nc.sync.dma_start(
                    out=h2t[:, k, :],
                    in_=rs_out[:, 128 * k:128 * k + 128].rearrange(
                        "i p -> p i"))
            nc.vector.tensor_add(h2t[:, :, :], h2t[:, :, :], b1t_sb)
            nc.scalar.activation(h2t[:, :, :], h2t[:, :, :], RELU)

            fc2_ps = f2ps.tile([4, 4], F32, tag="fc2ps")
            for k in range(4):
                nc.tensor.matmul(fc2_ps[:, :], lhsT=h2t[:, k, :],
                                 rhs=w2t_sb[:, k, :],
                                 start=(k == 0), stop=(k == 3))
            out_sb = wk.tile([4, NOUT], F32, tag="outsb")
            nc.vector.tensor_add(out_sb[:, :], fc2_ps[:, :], b2t_sb)
            nc.sync.dma_start(out=out_t[:, :], in_=out_sb[:, :])


def _get_program(stop_after: str = 'full'):
    key = ("prog_v2", stop_after)
    if key not in _CACHE:
        _CACHE[key] = _build_program(stop_after)
    return _CACHE[key]


def _feature_map():
    """Local (core-relative) original feature index for tiled weight slot
    (p, k).  Original local feature = cc*3136 + pix, pix = y*56 + x.
    Chunk i (K0[i] <= k < K0[i+1]): l = (k - K0[i])*128 + p =
    cc*PIX[i] + pos, pix = PIX0[i] + pos."""
    p = np.arange(128)[:, None]
    k = np.arange(NK)[None, :]
    m = np.empty((128, NK), np.int64)
    for i in range(3):
        kk = k[:, K0[i]:K0[i + 1]]
        l = (kk - K0[i]) * 128 + p
        cc, pos = l // PIX[i], l % PIX[i]
        m[:, K0[i]:K0[i + 1]] = cc * 3136 + PIX0[i] + pos
    return m


_FMAP = _feature_map()


def _host_prep(x, conv1_w, conv1_b, conv2_w, conv2_b, values, w_idx1,
               fc1_b, w_idx2, fc2_b):
    """Build per-core input maps (numpy, bf16 for PE-facing tensors)."""
    f32 = np.float32
    x = np.asarray(x, f32)
    conv1_w = np.asarray(conv1_w, f32)
    conv2_w = np.asarray(conv2_w, f32)
    values = np.asarray(values, f32)
    w_idx1 = np.asarray(w_idx1)
    w_idx2 = np.asarray(w_idx2)

    x_pad = np.zeros((B, 226, 232), f32)
    x_pad[:, 1:225, 1:225] = x[:, 0]

    # x9[c]: [72, 112, 224]; partition (dy*3+dx)*8 + h, h = half*4 + img_loc
    x9 = np.zeros((N_CORES, 72, PH, 224), f32)
    for dy in range(3):
        for dx in range(3):
            for h in range(8):
                half, il = h // 4, h % 4
                y0 = PH * half
                for c in range(N_CORES):
                    x9[c, (dy * 3 + dx) * 8 + h] = \
                        x_pad[4 * c + il, y0 + dy:y0 + dy + PH, dx:dx + 224]

    s1 = np.zeros((72, 128), f32)
    for dy in range(3):
        for dx in range(3):
            for h in range(8):
                s1[(dy * 3 + dx) * 8 + h, 16 * h:16 * h + C1] = \
                    conv1_w[:, 0, dy, dx]

    # conv2 stationaries [128, 6, 128]: pass t = dy*2 + grp;
    # partition p = e*64 + img*16 + ch supplies tap dx = 2*grp + e.
    # out column m = (oc//4)*16 + img*4 + oc%4 (a2a-friendly order).
    s2 = np.zeros((128, 6, 128), f32)
    for t in range(6):
        dy, grp = t // 2, t % 2
        for e in range(2):
            dx = 2 * grp + e
            if dx > 2:
                continue
            for img in range(4):
                for ch in range(C1):
                    for oc in range(C2):
                        s2[64 * e + 16 * img + ch, t,
                           (oc // 4) * 16 + img * 4 + oc % 4] = \
                            conv2_w[oc, ch, dy, dx]

    # fc1 weight: per core gather + permute to tiled feature order
    w1ts = []
    for c in range(N_CORES):
        idx = w_idx1[:, FSH * c:FSH * (c + 1)]          # [512, 12544]
        wcols = values[idx].T * WSCALE                   # [12544, 512]
        w1ts.append(np.ascontiguousarray(
            wcols[_FMAP]).astype(FP8NP))                 # [128, NK, 512]

    # packed consts [128, 38]: 0 cb1, 1 cb2, 2:18 b1t (k-major), 18:34 w2t
    # (w2t[p, k, o] = W2.T[k*128+p, o]), 34:38 b2t (partitions 0..3)
    cst = np.zeros((128, 38), f32)
    for h in range(8):
        cst[16 * h:16 * h + C1, 0] = np.asarray(conv1_b, f32)
    c2b = np.asarray(conv2_b, f32)
    for img in range(4):
        for oc in range(C2):
            cst[(oc // 4) * 16 + img * 4 + oc % 4, 1] = c2b[oc]
    cst[:, 2:18] = np.repeat(np.asarray(fc1_b, f32).reshape(4, 128).T,
                             4, axis=1) * WSCALE
    w2tT = values[w_idx2].T.astype(f32) / WSCALE         # [512, 4]
    cst[:, 18:34] = w2tT.reshape(4, 128, 4).transpose(1, 0, 2).reshape(128, 16)
    cst[:, 34:38] = np.asarray(fc2_b, f32)[None, :]

    s1 = s1.astype(BF16NP)
    s2 = s2.astype(BF16NP)
    in_maps = []
    for c in range(N_CORES):
        in_maps.append({
            "x9": np.ascontiguousarray(x9[c]).astype(BF16NP),
            "s1": s1, "s2": s2,
            "w1t": w1ts[c],
            "cst": cst,
        })
    return in_maps


def kernel(x, conv1_w, conv1_b, conv2_w, conv2_b, values, w_idx1, fc1_b,
           w_idx2, fc2_b, _trace=False, _trace_kwargs=None,
           _stop_after='full'):
    nc = _get_program(_stop_after)
    in_maps = _host_prep(x, conv1_w, conv1_b, conv2_w, conv2_b, values,
                         w_idx1, fc1_b, w_idx2, fc2_b)
    res = run_bass_kernel_spmd(nc, in_maps, core_ids=list(range(N_CORES)),
                               trace=_trace, **(_trace_kwargs or {}))
    out = np.zeros((B, NOUT), np.float32)
    for c in range(N_CORES):
        out[4 * c:4 * c + 4] = res.results[c]["out"]
    if _trace:
        kernel.last_result = res
    return out


if __name__ == "__main__":
    rng = np.random.default_rng(0)
    ins = {
        "x": rng.standard_normal((B, 1, IMG, IMG), dtype=np.float32),
        "conv1_w": rng.standard_normal((16, 1, 3, 3), dtype=np.float32) * 0.1,
        "conv1_b": np.zeros(16, np.float32),
        "conv2_w": rng.standard_normal((32, 16, 3, 3), dtype=np.float32) * 0.05,
        "conv2_b": np.zeros(32, np.float32),
        "values": np.sort(rng.standard_normal(4096).astype(np.float32) * 0.01),
        "w_idx1": rng.integers(0, 4096, (512, FEAT), dtype=np.int32),
        "fc1_b": np.zeros(512, np.float32),
        "w_idx2": rng.integers(0, 4096, (4, 512), dtype=np.int32),
        "fc2_b": np.zeros(4, np.float32),
    }
    out = kernel(**ins)
    print("out shape", out.shape, "sample row", out[0])


# revision 36
# speedup vs baseline: 1.5386x; 1.0258x over previous
"""Trainium2 Bass kernel for nn_MemristorCNN (embedding_lookup, 8 cores).

Strategy:
- Host gathers W1 = values[w_idx1], shards it column-wise over in_features
  (4 conv2 output channels per core), permutes columns to the device's
  chunked feature order and pre-tiles to [128, 100, 512] bf16 so the
  weight streams as 4 big contiguous SWDGE DMAs on the (otherwise idle)
  GpSimd queue, overlapping the whole conv stack.
- Conv stack is data-parallel (4 images/core).  conv1 packs
  (half, img, dy, dx) into K=72 with half in a contiguous 64-partition
  block so the pool1->conv2 repack is 8 large block DMAs.  conv2 packs
  (dx-pair, img, ch) into K=128 with 6 tap passes.
- PSUM evacuation does relu (+bias) first (fp32 PSUM -> bf16 SBUF,
  split between Scalar and Vector by a balance ratio), then 2x2 maxpool
  as two DVE tensor-max ops in bf16 2x mode (w pairs deinterleaved by
  the evacuation AP so operands are step-1).
- AllToAll runs in 2 row-chunks (rows 0:32, 32:56 + pad), each received
  buffer is [img, ch, pix] contiguous so one dma_start_transpose yields
  the feature-major fc1 operand; fc1 weight columns are host-permuted to
  the resulting (partition-major) feature order.  fc1 accumulates one
  PSUM group over 100 k-tiles; ReduceScatter + fc2 finish; host concats
  per-core [4, 4] outputs.
"""

import sys

import numpy as np
import ml_dtypes

BF16NP = ml_dtypes.bfloat16

for _p in ("/opt/trn_rl_repo",):
    if _p not in sys.path:
        sys.path.insert(0, _p)

import concourse.bacc as bacc
import concourse.bass as bass  # noqa: F401
import concourse.tile as tile
from concourse import mybir
from concourse.bass_utils import run_bass_kernel_spmd

F32 = mybir.dt.float32
FP8 = mybir.dt.float8e4
FP8NP = ml_dtypes.float8_e4m3
WSCALE = 256.0
BF16 = mybir.dt.bfloat16
RELU = mybir.ActivationFunctionType.Relu
COPY = mybir.ActivationFunctionType.Copy
ADD = mybir.AluOpType.add
MAX = mybir.AluOpType.max

N_CORES = 8
B = 32
IMG = 224
C1, C2 = 16, 32
PH, PW = 112, 112
HH, HW = 56, 56
FEAT = C2 * HH * HW          # 100352
FSH = FEAT // N_CORES        # 12544 = 4 ch * 3136 px
H1 = 512
NOUT = 4

# a2a row chunks (pooled rows 0:16, 16:36, 36:56) -> pix spans below;
# each chunk's 4ch*pix is a multiple of 128, so k-tiles pack exactly.
PIX = (896, 1120, 1120)          # px per channel per chunk
PIX0 = (0, 896, 2016)            # channel-relative px offset
CHT = (4, 9, 14)                 # conv2 T index after which chunk closes
NKC = tuple(4 * p // 128 for p in PIX)   # (28, 35, 35)
K0 = (0, NKC[0], NKC[0] + NKC[1], 98)
NK = 98
W_CHUNKS = 4

_CACHE = {}


def _build_program(stop_after: str = 'full'):
    nc = bacc.Bacc("TRN2", target_bir_lowering=False, debug=False,
                   num_devices=N_CORES)
    _emit(nc, stop_after)
    nc.compile()
    return nc


def _emit(nc, stop_after: str):
    # ---- kernel I/O ----
    x9_t = nc.dram_tensor("x9", [72, PH, 224], BF16, kind="ExternalInput")
    s1_t = nc.dram_tensor("s1", [72, 128], BF16, kind="ExternalInput")
    s2_t = nc.dram_tensor("s2", [128, 6, 128], BF16, kind="ExternalInput")
    w1t_t = nc.dram_tensor("w1t", [128, NK, H1], FP8, kind="ExternalInput")
    # packed small consts: col 0 cb1, 1 cb2, 2:18 b1t, 18:34 w2t, 34:38 b2t
    cst_t = nc.dram_tensor("cst", [128, 38], F32, kind="ExternalInput")
    out_t = nc.dram_tensor("out", [4, NOUT], F32, kind="ExternalOutput")

    # ---- internal DRAM (collective bounce buffers) ----
    a2a_ins = [nc.dram_tensor(f"a2a_in{i}", [B, 4, PIX[i]], BF16)
               for i in range(3)]
    a2a_outs = [nc.dram_tensor(f"a2a_out{i}", [B, 4, PIX[i]], BF16)
                for i in range(3)]
    rs_in = nc.dram_tensor("rs_in", [B, H1], F32)
    rs_out = nc.dram_tensor("rs_out", [4, H1], F32)
    warm_in = nc.dram_tensor("warm_in", [8, 64], BF16)
    warm_out = nc.dram_tensor("warm_out", [8, 64], BF16)

    groups = [list(range(N_CORES))]

    with tile.TileContext(nc) as tc:
        with (
            tc.tile_pool(name="wgt", bufs=1) as wgt,
            tc.tile_pool(name="const", bufs=1) as cpool,
            tc.tile_pool(name="pers", bufs=1) as pers,
            tc.tile_pool(name="xq", bufs=2) as xq,
            tc.tile_pool(name="ev", bufs=2) as ev,
            tc.tile_pool(name="wk", bufs=2) as wk,
            tc.tile_pool(name="cps", bufs=3, space="PSUM") as cps,
            tc.tile_pool(name="fps", bufs=1, space="PSUM") as fps,
        ):
            # -------- latency-critical loads first --------
            # small consts first (conv1 stationaries), then the two x9
            # halves on the two separate HWDGE rings in parallel.
            s1_sb = cpool.tile([72, 128], BF16, tag="s1")
            nc.scalar.dma_start(out=s1_sb[:, :], in_=s1_t[:, :])
            s2_sb = cpool.tile([128, 6, 128], BF16, tag="s2")
            nc.scalar.dma_start(out=s2_sb[:, :, :], in_=s2_t[:, :, :])
            cst_sb = cpool.tile([128, 38], F32, tag="cst")
            nc.scalar.dma_start(out=cst_sb[:, :], in_=cst_t[:, :])
            x9_tiles = []
            for q in range(2):
                x9q = xq.tile([72, 56, 224], BF16, tag="x9")
                eng = nc.scalar if q == 0 else nc.sync
                eng.dma_start(out=x9q[:, :, :],
                              in_=x9_t[:, 56 * q:56 * q + 56, :])
                x9_tiles.append(x9q)

            # PE warm-up: garbage matmuls so the HAM clock-gate opens
            # before conv1's first real matmul (and stays open).
            junk = cpool.tile([72, 512], BF16, tag="junk")
            nc.vector.memset(junk[:, :], 0.0)
            warm_psa = fps.tile([B, H1], F32, tag="fc1psa")
            warm_psb = fps.tile([B, H1], F32, tag="fc1psb")
            for i in range(12):
                wp = warm_psa if i % 2 == 0 else warm_psb
                nc.tensor.matmul(wp[:, :], lhsT=s1_sb[:, 0:32],
                                 rhs=junk[:, :], start=True, stop=True)
            cb1_sb = cst_sb[:, 0:1]
            cb2_sb = cst_sb[:, 1:2]
            b1t_sb = cst_sb[:, 2:18].rearrange("p (k i) -> p k i", k=4)
            w2t_sb = cst_sb[:, 18:34].rearrange("p (k o) -> p k o", k=4)
            b2t_sb = cst_sb[0:4, 34:38]

            # warm up the collective path so the first real a2a runs at
            # full rate (the first collective pays ~45us of setup); the
            # input is never initialized -- nothing reads the result.
            nc.gpsimd.collective_compute(
                "AllToAll", mybir.AluOpType.bypass, replica_groups=groups,
                ins=[warm_in[:, :]], outs=[warm_out[:, :]])

            # conv2 input: partition e*64 + img*16 + ch; rows 1+g (g = global
            # pooled conv1 row), dx-shifted by e.  Zero only the halo border.
            c2in = pers.tile([128, 114, 116], BF16, tag="c2in")
            nc.gpsimd.memset(c2in[:, 0, :], 0.0)
            nc.gpsimd.memset(c2in[:, 113, :], 0.0)
            nc.gpsimd.memset(c2in[0:64, :, 0:1], 0.0)
            nc.gpsimd.memset(c2in[0:64, :, 113:116], 0.0)
            nc.gpsimd.memset(c2in[64:128, :, 112:116], 0.0)

            # -------- fc1 weight stream on GpSimd (SWDGE), 4 big chunks ----
            # (held behind x9 q0/q1 arrival so the input loads win the HBM
            # bandwidth race at startup)
            w_sb = wgt.tile([128, NK, H1], FP8, tag="w1")
            wgate = wk.tile([1, 8], F32, tag="wgate")
            nc.gpsimd.tensor_copy(wgate[:, 0:1], x9_tiles[0][0:1, 0, 0:1])
            nc.gpsimd.tensor_copy(wgate[:, 1:2], x9_tiles[1][0:1, 0, 0:1])
            nc.gpsimd.tensor_copy(wgate[:, 2:3], s1_sb[0:1, 0:1])
            nc.gpsimd.tensor_copy(wgate[:, 3:4], s2_sb[0:1, 0, 0:1])
            nc.gpsimd.tensor_copy(wgate[:, 4:5], cst_sb[0:1, 0:1])
            kb = [0, 25, 50, 75, NK]
            for ci in range(W_CHUNKS):
                # write one element of the chunk's region first (reading the
                # gate) so the big DMA has a true data dependency on the
                # startup loads and cannot be scheduled before them.
                nc.gpsimd.tensor_copy(w_sb[0:1, kb[ci]:kb[ci] + 1, 0:1],
                                      wgate[0:1, ci:ci + 1])
                nc.gpsimd.dma_start(out=w_sb[:, kb[ci]:kb[ci + 1], :],
                                    in_=w1t_t[:, kb[ci]:kb[ci + 1], :])

            # ---------------- conv1 + relu + pool1 ----------------
            # out partition m = half*64 + img*16 + oc.  The matmul writes
            # PSUM through a strided AP so the psum layout is [g, r, w, x]
            # (pool pairs deinterleaved); evacuation is then a contiguous
            # copy and the pool maxes run in DVE 2x bf16 mode.  The pooled
            # output goes straight into c2in: half0 -> e0 block, half1 ->
            # e1 block (same partitions); the two cross copies are DMAs.

            def evac(out_c, ps_v, bias, use_vector):
                # relu(+bias): fp32 PSUM -> bf16 SBUF, contiguous
                if use_vector:
                    nc.vector.tensor_scalar(out_c, ps_v, bias, 0.0,
                                            op0=ADD, op1=MAX)
                else:
                    nc.scalar.activation(out_c, ps_v, RELU, bias=bias)

            for T in range(28):            # 2 pooled rows per psum tile
                ps = cps.tile([128, 2, 512], F32, tag="cps")
                for g in range(2):
                    yp = T * 2 + g         # pooled row within half
                    q, ypl = yp // 28, yp % 28
                    nc.tensor.matmul(
                        ps[:, g, 0:448],
                        lhsT=s1_sb[:, :],
                        rhs=x9_tiles[q][:, 2 * ypl:2 * ypl + 2, :],
                        start=True, stop=True)
                # evac reads psum via a w-deinterleaving view; c1 is written
                # contiguous as [w, g, r, x]
                c1 = ev.tile([128, 2, 2, 2, 112], BF16, tag="c1")
                evac(c1[:, :, :, :, :],
                     ps[:, :, 0:448].rearrange("p g (r x w) -> p w g r x",
                                               r=2, w=2),
                     cb1_sb, use_vector=(T % 14 >= 12))
                m1 = ev.tile([128, 2, 2, 112], BF16, tag="m1")     # [g,r,x]
                nc.vector.tensor_max(m1[:, :, :, :], c1[:, 0, :, :, :],
                                     c1[:, 1, :, :, :])
                # pooled rows 2T, 2T+1: half0 -> c2in e0 rows 1+2T,
                # half1 -> c2in e1 rows 57+2T
                nc.vector.tensor_max(
                    c2in[0:64, 1 + 2 * T:3 + 2 * T, 1:113],
                    m1[0:64, :, 0, :], m1[0:64, :, 1, :])
                nc.vector.tensor_max(
                    c2in[64:128, 57 + 2 * T:59 + 2 * T, 0:112],
                    m1[64:128, :, 0, :], m1[64:128, :, 1, :])

                if T == 13 or T == 27:
                    # cross copies for rows chunk (28 rows each half):
                    # half0 block (e0, rows 1..57) -> e1 partitions;
                    # half1 block (e1, rows 57..113) -> e0 partitions.
                    r0 = 1 + 28 * (T // 14)
                    r1 = 57 + 28 * (T // 14)
                    nc.sync.dma_start(
                        out=c2in[64:128, r0:r0 + 28, 0:112],
                        in_=c2in[0:64, r0:r0 + 28, 1:113])
                    nc.sync.dma_start(
                        out=c2in[0:64, r1:r1 + 28, 1:113],
                        in_=c2in[64:128, r1:r1 + 28, 0:112])

            if stop_after == "conv1":
                dbg = wk.tile([4, NOUT], F32, tag="outsb")
                nc.vector.tensor_copy(dbg[:, :], c2in[0:4, 1, 1:5])
                nc.sync.dma_start(out=out_t[:, :], in_=dbg[:, :])
                return

            # ---------------- conv2 + relu + pool2 ----------------
            # out partition m = img*32 + oc; 6 passes t=(dy, grp):
            # partition block e supplies tap dx = 2*grp + e.
            h_sb = pers.tile([128, 14, 4, 56], BF16, tag="hsb")  # [T, row, x]
            hT = pers.tile([128, NK, B], BF16, tag="hT")
            for T in range(14):            # 8 conv rows / 4 pooled rows
                ps = cps.tile([128, 2, 512], F32, tag="cps")
                for t in range(6):
                    dy, grp = t // 2, t % 2
                    for sub in range(2):   # alternate psum banks
                        y0 = 8 * T + 4 * sub
                        nc.tensor.matmul(
                            ps[:, sub, 0:448],
                            lhsT=s2_sb[:, t, :],
                            rhs=c2in[:, y0 + dy:y0 + dy + 4,
                                     2 * grp:2 * grp + 112],
                            start=(t == 0), stop=(t == 5))
                c1 = ev.tile([128, 2, 2, 4, 56], BF16, tag="c2c1")  # [w,s,r,x]
                evac(c1[:, :, :, :, :],
                     ps[:, :, 0:448].rearrange("p s (r x w) -> p w s r x",
                                               r=4, w=2),
                     cb2_sb, use_vector=(T % 14 >= 11))
                m1 = ev.tile([128, 2, 4, 56], BF16, tag="c2m1")     # [s,r,x]
                nc.vector.tensor_max(m1[:, :, :, :], c1[:, 0, :, :, :],
                                     c1[:, 1, :, :, :])
                v2 = m1[:, :, :, :].rearrange("p s (rp rw) x -> p s rp rw x",
                                              rw=2)
                nc.vector.tensor_max(h_sb[:, T, :, :].rearrange(
                    "p (s rp) x -> p s rp x", s=2),
                    v2[:, :, :, 0, :], v2[:, :, :, 1, :])

                # -------- a2a chunks fire as their rows complete --------
                # h partition m = 16d + 4j + c  <->  dst row (4d+j) ch c,
                # so each write is a plain contiguous 2D copy.  The
                # transpose to feature-major (hT[p, k, i] = chunk feature
                # k*128+p) follows each collective immediately.
                if T + 1 in CHT:
                    i = CHT.index(T + 1)
                    Ta, Tb = ((0,) + CHT)[i], CHT[i]
                    nc.sync.dma_start(
                        out=a2a_ins[i][:, :, :].rearrange(
                            "r c x -> (r c) x"),
                        in_=h_sb[:, Ta:Tb, :, :].rearrange(
                            "p a b x -> p (a b x)"))
                    nc.gpsimd.collective_compute(
                        "AllToAll", mybir.AluOpType.bypass,
                        replica_groups=groups,
                        ins=[a2a_ins[i][:, :, :]],
                        outs=[a2a_outs[i][:, :, :]])
                    nc.sync.dma_start_transpose(
                        out=hT[:, K0[i]:K0[i + 1], :],
                        in_=a2a_outs[i][:, :, :].rearrange(
                            "i c x -> i (c x)"))

            if stop_after == "conv2":
                dbg = wk.tile([4, NOUT], F32, tag="outsb")
                nc.vector.tensor_copy(dbg[:, :], h_sb[0:4, 0, 0, 0:4])
                nc.sync.dma_start(out=out_t[:, :], in_=dbg[:, :])
                return

            if stop_after == "a2a":
                dbg = wk.tile([4, NOUT], F32, tag="outsb")
                nc.vector.tensor_copy(dbg[:, :], hT[0:4, 0, 0:4])
                nc.sync.dma_start(out=out_t[:, :], in_=dbg[:, :])
                return

            # ---------------- fc1 partial ----------------
            # two psum banks, alternated per k-tile so consecutive matmuls
            # never chain into the same bank (fill/drain overlap)
            fc1_psa = fps.tile([B, H1], F32, tag="fc1psa")
            fc1_psb = fps.tile([B, H1], F32, tag="fc1psb")
            for k in range(NK):
                pp = fc1_psa if k % 2 == 0 else fc1_psb
                nc.tensor.matmul(pp[:, :], lhsT=hT[:, k, :],
                                 rhs=w_sb[:, k, :],
                                 start=(k < 2), stop=(k >= NK - 2))
            fc1_sb = wk.tile([B, H1], F32, tag="fc1")
            nc.scalar.activation(fc1_sb[:, :], fc1_psa[:, :], COPY)
            nc.vector.tensor_add(fc1_sb[:, :], fc1_sb[:, :], fc1_psb[:, :])
            nc.sync.dma_start(out=rs_in[:, :], in_=fc1_sb[:, :])

            if stop_after == "fc1":
                nc.sync.dma_start(out=out_t[:, :], in_=fc1_sb[0:4, 0:4])
                return

            # -------- ReduceScatter + bias + relu + fc2 --------
            nc.gpsimd.collective_compute(
                "ReduceScatter", mybir.AluOpType.add, replica_groups=groups,
                ins=[rs_in[:, :]], outs=[rs_out[:, :]])

            h2t = wk.tile([128, 4, 4], F32, tag="h2t")   # [c, k, img]
            for k in range(4):
                nc.sync.dma_start(
                    out=h2t[:, k, :],
                    in_=rs_out[:, 128 * k:128 * k + 128].rearrange(
                        "i p -> p i"))
            nc.vector.tensor_add(h2t[:, :, :], h2t[:, :, :], b1t_sb)
            nc.scalar.activation(h2t[:, :, :], h2t[:, :, :], RELU)

            fc2_big = cps.tile([128, 2, 512], F32, tag="cps")
            fc2_ps = fc2_big[0:4, 0, 0:4]
            for k in range(4):
                nc.tensor.matmul(fc2_ps, lhsT=h2t[:, k, :],
                                 rhs=w2t_sb[:, k, :],
                                 start=(k == 0), stop=(k == 3))
            out_sb = wk.tile([4, NOUT], F32, tag="outsb")
            nc.vector.tensor_add(out_sb[:, :], fc2_ps, b2t_sb)
            nc.sync.dma_start(out=out_t[:, :], in_=out_sb[:, :])


def _get_program(stop_after: str = 'full'):
    key = ("prog_v2", stop_after)
    if key not in _CACHE:
        _CACHE[key] = _build_program(stop_after)
    return _CACHE[key]


def _feature_map():
    """Local (core-relative) original feature index for tiled weight slot
    (p, k).  Original local feature = cc*3136 + pix, pix = y*56 + x.
    Chunk i (K0[i] <= k < K0[i+1]): l = (k - K0[i])*128 + p =
    cc*PIX[i] + pos, pix = PIX0[i] + pos."""
    p = np.arange(128)[:, None]
    k = np.arange(NK)[None, :]
    m = np.empty((128, NK), np.int64)
    for i in range(3):
        kk = k[:, K0[i]:K0[i + 1]]
        l = (kk - K0[i]) * 128 + p
        cc, pos = l // PIX[i], l % PIX[i]
        m[:, K0[i]:K0[i + 1]] = cc * 3136 + PIX0[i] + pos
    return m


_FMAP = _feature_map()


def _host_prep(x, conv1_w, conv1_b, conv2_w, conv2_b, values, w_idx1,
               fc1_b, w_idx2, fc2_b):
    """Build per-core input maps (numpy, bf16 for PE-facing tensors)."""
    f32 = np.float32
    x = np.asarray(x, f32)
    conv1_w = np.asarray(conv1_w, f32)
    conv2_w = np.asarray(conv2_w, f32)
    values = np.asarray(values, f32)
    w_idx1 = np.asarray(w_idx1)
    w_idx2 = np.asarray(w_idx2)

    x_pad = np.zeros((B, 226, 232), f32)
    x_pad[:, 1:225, 1:225] = x[:, 0]

    # x9[c]: [72, 112, 224]; partition (dy*3+dx)*8 + h, h = half*4 + img_loc
    x9 = np.zeros((N_CORES, 72, PH, 224), f32)
    for dy in range(3):
        for dx in range(3):
            for h in range(8):
                half, il = h // 4, h % 4
                y0 = PH * half
                for c in range(N_CORES):
                    x9[c, (dy * 3 + dx) * 8 + h] = \
                        x_pad[4 * c + il, y0 + dy:y0 + dy + PH, dx:dx + 224]

    s1 = np.zeros((72, 128), f32)
    for dy in range(3):
        for dx in range(3):
            for h in range(8):
                s1[(dy * 3 + dx) * 8 + h, 16 * h:16 * h + C1] = \
                    conv1_w[:, 0, dy, dx]

    # conv2 stationaries [128, 6, 128]: pass t = dy*2 + grp;
    # partition p = e*64 + img*16 + ch supplies tap dx = 2*grp + e.
    # out column m = (oc//4)*16 + img*4 + oc%4 (a2a-friendly order).
    s2 = np.zeros((128, 6, 128), f32)
    for t in range(6):
        dy, grp = t // 2, t % 2
        for e in range(2):
            dx = 2 * grp + e
            if dx > 2:
                continue
            for img in range(4):
                for ch in range(C1):
                    for oc in range(C2):
                        s2[64 * e + 16 * img + ch, t,
                           (oc // 4) * 16 + img * 4 + oc % 4] = \
                            conv2_w[oc, ch, dy, dx]

    # fc1 weight: per core gather + permute to tiled feature order
    w1ts = []
    for c in range(N_CORES):
        idx = w_idx1[:, FSH * c:FSH * (c + 1)]          # [512, 12544]
        wcols = values[idx].T * WSCALE                   # [12544, 512]
        w1ts.append(np.ascontiguousarray(
            wcols[_FMAP]).astype(FP8NP))                 # [128, NK, 512]

    # packed consts [128, 38]: 0 cb1, 1 cb2, 2:18 b1t (k-major), 18:34 w2t
    # (w2t[p, k, o] = W2.T[k*128+p, o]), 34:38 b2t (partitions 0..3)
    cst = np.zeros((128, 38), f32)
    for h in range(8):
        cst[16 * h:16 * h + C1, 0] = np.asarray(conv1_b, f32)
    c2b = np.asarray(conv2_b, f32)
    for img in range(4):
        for oc in range(C2):
            cst[(oc // 4) * 16 + img * 4 + oc % 4, 1] = c2b[oc]
    cst[:, 2:18] = np.repeat(np.asarray(fc1_b, f32).reshape(4, 128).T,
                             4, axis=1) * WSCALE
    w2tT = values[w_idx2].T.astype(f32) / WSCALE         # [512, 4]
    cst[:, 18:34] = w2tT.reshape(4, 128, 4).transpose(1, 0, 2).reshape(128, 16)
    cst[:, 34:38] = np.asarray(fc2_b, f32)[None, :]

    s1 = s1.astype(BF16NP)
    s2 = s2.astype(BF16NP)
    in_maps = []
    for c in range(N_CORES):
        in_maps.append({
            "x9": np.ascontiguousarray(x9[c]).astype(BF16NP),
            "s1": s1, "s2": s2,
            "w1t": w1ts[c],
            "cst": cst,
        })
    return in_maps


def kernel(x, conv1_w, conv1_b, conv2_w, conv2_b, values, w_idx1, fc1_b,
           w_idx2, fc2_b, _trace=False, _trace_kwargs=None,
           _stop_after='full'):
    nc = _get_program(_stop_after)
    in_maps = _host_prep(x, conv1_w, conv1_b, conv2_w, conv2_b, values,
                         w_idx1, fc1_b, w_idx2, fc2_b)
    res = run_bass_kernel_spmd(nc, in_maps, core_ids=list(range(N_CORES)),
                               trace=_trace, **(_trace_kwargs or {}))
    out = np.zeros((B, NOUT), np.float32)
    for c in range(N_CORES):
        out[4 * c:4 * c + 4] = res.results[c]["out"]
    if _trace:
        kernel.last_result = res
    return out


if __name__ == "__main__":
    rng = np.random.default_rng(0)
    ins = {
        "x": rng.standard_normal((B, 1, IMG, IMG), dtype=np.float32),
        "conv1_w": rng.standard_normal((16, 1, 3, 3), dtype=np.float32) * 0.1,
        "conv1_b": np.zeros(16, np.float32),
        "conv2_w": rng.standard_normal((32, 16, 3, 3), dtype=np.float32) * 0.05,
        "conv2_b": np.zeros(32, np.float32),
        "values": np.sort(rng.standard_normal(4096).astype(np.float32) * 0.01),
        "w_idx1": rng.integers(0, 4096, (512, FEAT), dtype=np.int32),
        "fc1_b": np.zeros(512, np.float32),
        "w_idx2": rng.integers(0, 4096, (4, 512), dtype=np.int32),
        "fc2_b": np.zeros(4, np.float32),
    }
    out = kernel(**ins)
    print("out shape", out.shape, "sample row", out[0])
